# revision 42
# baseline (speedup 1.0000x reference)
"""Trainium2 Bass kernel for CachedMultiHeadAttention.

Problem: B=16, Q=32, KV=4096, D=1024, H=16 (DH=64), fp32 in/out.

Sharding: TP-4 x DP-2 hybrid. Core (dp, tp) handles batches [8dp, 8dp+8)
and heads [4tp, 4tp+4) = head-dim columns [256tp, 256tp+256).
Wq/Wk/Wv are column-split, Wo is row-split; each core emits a full-width
partial y for its 8 batches and the host sums the 4 TP partials per DP
group (and adds bo once). This cuts per-core weight DMA 16.8 MB -> 4.2 MB
while the mandatory 64 MiB/core KV stream stays untouched.

Per-core dataflow (per batch: 4 heads, 256 head-dims, 32 queries):
  - x^T via PE transpose; q held as per-batch block-diagonal fp16 moving
    operands (2 heads per [128, 64] tile).
  - K stripes [512, 256] stream in f32, PE-transposed (f32r, 1.5 cyc/row)
    to fp16 K^T one stripe ahead of use.
  - Scores are computed TRANSPOSED: scT[s, hq] = kt_chunk^T @ qbd with
    K^T stationary. exp(scT) lands directly in W@V-stationary layout, so
    there is no per-stripe W transpose or extra PSUM->SBUF copy on the
    critical chain.
  - V converts f32->fp16 on the otherwise-idle GpSimd engine; W@V runs
    fp16 x fp16 with a ones-column giving the softmax denominator in
    column 256 of the accumulator.
  - Per-batch: normalize, transpose to wv^T, project y^T = Wo_rows^T @ wv^T
    (no bias), transpose back, store 32 rows of the partial y. Streaming
    the output per batch keeps the post-DMA tail to a few microseconds.
"""

import ml_dtypes
import numpy as np

import concourse.bass as bass
import concourse.bacc as bacc
import concourse.mybir as mybir
import concourse.tile as tile
from concourse.bass_utils import run_bass_kernel_spmd
from concourse.masks import make_identity

F32 = mybir.dt.float32
F32R = mybir.dt.float32r
BF16 = mybir.dt.bfloat16
FP16 = mybir.dt.float16

B, Q, KV, D, H = 16, 32, 4096, 1024, 16
DH = D // H                     # 64
NCORES = 8
NTP = 4                         # tensor-parallel ways (heads)
NDP = 2                         # data-parallel ways (batch)
NB = B // NDP                   # 8 batches per core
HPC = H // NTP                  # 4 heads per core
DSL = D // NTP                  # 256 head-dim slice per core
TOK = NB * Q                    # 256 tokens per core
SCALE = float(DH) ** -0.5       # folded q*k scale (DH**-0.25 applied twice)
NSTRIPE = 8                     # stripes of 512 cached s positions per batch
STRIPE = 512
GW = 260                        # v_aug stride (256 V + 2 ones + 2 pad)
NWV = 258                       # W@V moving size: 256 V cols + ones + dup


def _build_kernel():
    nc = bacc.Bacc(
        "TRN2",
        target_bir_lowering=False,
        debug=False,
        enable_asserts=False,
        num_devices=NCORES,
    )

    x_d = nc.dram_tensor("x", [TOK, D], F32R, kind="ExternalInput").ap()
    ck_d = nc.dram_tensor("cache_k", [NB, KV, DSL], F32R, kind="ExternalInput").ap()
    cv_d = nc.dram_tensor("cache_v", [NB, KV, DSL], F32R, kind="ExternalInput").ap()
    wq_d = nc.dram_tensor("Wq", [D, DSL], F32R, kind="ExternalInput").ap()
    wk_d = nc.dram_tensor("Wk", [D, DSL], BF16, kind="ExternalInput").ap()
    wv_d = nc.dram_tensor("Wv", [D, DSL], BF16, kind="ExternalInput").ap()
    wo_d = nc.dram_tensor("Wo", [DSL, D], BF16, kind="ExternalInput").ap()
    bq_d = nc.dram_tensor("bq", [DSL], F32, kind="ExternalInput").ap()
    bv_d = nc.dram_tensor("bv", [DSL], F32, kind="ExternalInput").ap()
    y_d = nc.dram_tensor("y", [NB, 128, 256], BF16, kind="ExternalOutput").ap()

    with tile.TileContext(nc) as tc:
        _body(tc, x_d, ck_d, cv_d, wq_d, wk_d, wv_d, wo_d, bq_d, bv_d, y_d)
    nc.compile()
    return nc


def _trT(nc, out_ap, in_ap, ident_r):
    """PE transpose in f32r (1.5 cyc/row vs 2.0 for plain f32). The input
    must come from a producer that wrote f32r (DMA / copy into an f32r
    tile) or the BIR verifier rejects the NEFF."""
    nc.tensor.matmul(
        out_ap.bitcast(F32R), in_ap.bitcast(F32R), ident_r,
        start=True, stop=True, is_transpose=True,
    )


def _trF(nc, out_ap, in_ap, ident):
    """Plain f32 PE transpose (2 cyc/row) for f32-produced data."""
    nc.tensor.matmul(
        out_ap, in_ap, ident, start=True, stop=True, is_transpose=True,
    )


def _body(tc, x_d, ck_d, cv_d, wq_d, wk_d, wv_d, wo_d, bq_d, bv_d, y_d):
    nc = tc.nc
    Exp = mybir.ActivationFunctionType.Exp

    with (
        tc.tile_pool(name="consts", bufs=1) as consts,
        tc.tile_pool(name="wo_pool", bufs=1) as wo_pool,
    ):
        identity = consts.tile([128, 128], F32)
        make_identity(nc, identity)
        ident_r = consts.tile([128, 128], F32R)
        nc.vector.tensor_copy(ident_r, identity)
        ones_row = consts.tile([1, TOK], F32)
        nc.vector.memset(ones_row, 1.0)

        bq_sb = consts.tile([1, DSL], F32)
        bv_sb = consts.tile([1, DSL], F32)

        x_sb = consts.tile([128, 2, D], F32R)
        # DMA issue order on the SP queue: x, Wq first (q projection gates
        # the first QK), then KV stripes stream from the main loop.
        nc.sync.dma_start(out=x_sb, in_=x_d.rearrange("(c p) d -> p c d", p=128))

        wo_sb = wo_pool.tile([128, 2, D], BF16)

        xT = consts.tile([128, 8, TOK], F32R)    # [d-part, d-chunk, tok]
        xT_bf = consts.tile([128, 8, TOK], BF16) # bf16 view for k/v proj
        # block-diagonal fp16 q^T: chunk 2b+m holds batch b, head pair m:
        # rows 0:64 x cols 0:32 = even head, rows 64:128 x cols 32:64 = odd.
        qbd = consts.tile([128, 2 * NB, 64], FP16)
        kTc = consts.tile([128, 2, TOK], FP16)   # current-token K^T
        wvT = consts.tile([128, 2, TOK], BF16)   # attention output, transposed
        v_cur = consts.tile([Q, NB, GW], F32R)   # V_aug for current tokens

        wq_sb = consts.tile([128, 8, DSL], F32R)
        wk_sb = consts.tile([128, 8, DSL], BF16)
        wv_sb = consts.tile([128, 8, DSL], BF16)
        vT_sb = consts.tile([128, 2, TOK], F32R)

        # ---------------- stage A: x^T and q projection ----------------
        with (
            tc.tile_pool(name="ppsum", bufs=3, space="PSUM") as ppsum,
        ):
            # SP queue: x, Wq, bq gate the q projection and the stripe QKs;
            # the other weights ride the scalar queue and interleave into
            # the stripe stream (their consumers run mid-loop).
            nc.sync.dma_start(out=wq_sb, in_=wq_d.rearrange("(c p) m -> p c m", p=128))
            nc.sync.dma_start(out=bq_sb, in_=bq_d.rearrange("(a d) -> a d", a=1))
            nc.scalar.dma_start(out=wk_sb, in_=wk_d.rearrange("(c p) m -> p c m", p=128))
            nc.scalar.dma_start(out=wv_sb, in_=wv_d.rearrange("(c p) m -> p c m", p=128))
            nc.scalar.dma_start(out=wo_sb, in_=wo_d.rearrange("(c p) d -> p c d", p=128))
            nc.scalar.dma_start(out=bv_sb, in_=bv_d.rearrange("(a d) -> a d", a=1))

            # warmup op: first PE instruction depends only on the gpsimd
            # identity, so real work never accumulates a Pool wait.
            warm_ps = ppsum.tile([128, TOK], F32, tag="pp")
            nc.tensor.matmul(
                warm_ps[0:1, 0:1], identity[:, 0:1], identity[:, 0:1],
                start=True, stop=True,
            )
            for k in range(8):
                xt_ps = ppsum.tile([128, TOK], F32, tag="pp")
                for c in range(2):
                    _trT(nc, xt_ps[:, 128 * c : 128 * c + 128],
                         x_sb[:, c, 128 * k : 128 * k + 128], ident_r)
                nc.scalar.copy(out=xT[:, k, :], in_=xt_ps)

            nc.vector.memset(qbd, 0.0)
            for m in range(2):
                qp = ppsum.tile([128, TOK], F32, tag="pp")
                for k in range(8):
                    nc.tensor.matmul(
                        qp, wq_sb[:, k, 128 * m : 128 * m + 128], xT[:, k, :],
                        start=(k == 0), stop=False,
                    )
                nc.tensor.matmul(
                    qp, bq_sb[0:1, 128 * m : 128 * m + 128], ones_row,
                    start=False, stop=True,
                )
                for b in range(NB):
                    eng = nc.scalar if b % 2 == 0 else nc.vector
                    if b % 2 == 0:
                        nc.scalar.copy(
                            out=qbd[0:64, 2 * b + m, 0:Q],
                            in_=qp[0:64, Q * b : Q * b + Q],
                        )
                        nc.scalar.copy(
                            out=qbd[64:128, 2 * b + m, Q : 2 * Q],
                            in_=qp[64:128, Q * b : Q * b + Q],
                        )
                    else:
                        nc.vector.tensor_copy(
                            qbd[0:64, 2 * b + m, 0:Q],
                            qp[0:64, Q * b : Q * b + Q],
                        )
                        nc.vector.tensor_copy(
                            qbd[64:128, 2 * b + m, Q : 2 * Q],
                            qp[64:128, Q * b : Q * b + Q],
                        )


        # ---------------- main attention loop ----------------
        with (
            tc.tile_pool(name="knat", bufs=10) as knat_p,
            tc.tile_pool(name="vaug", bufs=8) as vaug_p,
            tc.tile_pool(name="ktp", bufs=2) as kt_p,
            tc.tile_pool(name="work", bufs=3) as work,
            tc.tile_pool(name="spsum", bufs=2, space="PSUM") as spsum,
            tc.tile_pool(name="trpsum", bufs=2, space="PSUM") as trpsum,
            tc.tile_pool(name="opsum", bufs=2, space="PSUM") as opsum,
            tc.tile_pool(name="ypsum", bufs=1, space="PSUM") as ypsum,
            tc.tile_pool(name="ftrp", bufs=1, space="PSUM") as ftrp,
        ):
            ck_r = [ck_d[b].rearrange("(j p) d -> p j d", p=128) for b in range(NB)]
            cv_r = [cv_d[b].rearrange("(j p) d -> p j d", p=128) for b in range(NB)]
            NG = NB * NSTRIPE          # 64 global stripes

            def dma_k(g):
                b, S = divmod(g, NSTRIPE)
                k_nat = knat_p.tile([128, 4, DSL], F32R, tag="k")
                nc.sync.dma_start(out=k_nat, in_=ck_r[b][:, 4 * S : 4 * S + 4, :])
                return k_nat

            def dma_v(g):
                b, S = divmod(g, NSTRIPE)
                v_aug = vaug_p.tile([128, 4, GW], F32R, tag="v")
                nc.vector.memset(v_aug[:, :, 256:258].bitcast(F32), 1.0)
                if g == NB * NSTRIPE - 1:
                    # split the final V transfer 3+1 so the last chunk's
                    # W@V chain after the final bytes is minimal
                    nc.sync.dma_start(
                        out=v_aug[:, 0:3, 0:256],
                        in_=cv_r[b][:, 4 * S : 4 * S + 3, :],
                    )
                    nc.sync.dma_start(
                        out=v_aug[:, 3:4, 0:256],
                        in_=cv_r[b][:, 4 * S + 3 : 4 * S + 4, :],
                    )
                else:
                    nc.sync.dma_start(
                        out=v_aug[:, :, 0:256],
                        in_=cv_r[b][:, 4 * S : 4 * S + 4, :],
                    )
                return v_aug

            def prep_kt(k_nat):
                """PE-transpose a K stripe into fp16 K^T (one stripe ahead)."""
                kt = kt_p.tile([128, 2, STRIPE], FP16, tag="kt")
                for dc in range(2):
                    tr_ps = trpsum.tile([128, STRIPE], F32, tag="tr")
                    for jj in range(4):
                        _trT(nc, tr_ps[:, 128 * jj : 128 * jj + 128],
                             k_nat[:, jj, 128 * dc : 128 * dc + 128], ident_r)
                    if dc == 0:
                        nc.scalar.copy(out=kt[:, dc, :], in_=tr_ps)
                    else:
                        nc.vector.tensor_copy(kt[:, dc, :], tr_ps)
                return kt

            # prologue: K two stripes ahead, V one ahead
            kpend = {0: dma_k(0), 1: dma_k(1)}
            v16 = dma_v(0)
            kt = prep_kt(kpend.pop(0))

            def cur_attn(cb, co_ps, opens):
                scT_c = spsum.tile([128, STRIPE], F32, tag="sc")
                for m in range(2):
                    nc.tensor.matmul(
                        scT_c[0:Q, 64 * m : 64 * m + 64],
                        kTc[:, m, Q * cb : Q * cb + Q],
                        qbd[:, 2 * cb + m, :],
                        start=True, stop=True,
                    )
                wT_c = work.tile([Q, 128], F32R, tag="wT_c")
                nc.scalar.activation(wT_c, scT_c[0:Q, 0:128], Exp, scale=SCALE)
                nc.tensor.matmul(
                    co_ps, wT_c, v_cur[:, cb, 0:NWV],
                    start=opens, stop=not opens, skip_group_check=True,
                )

            def proj_kv(m):
                """k^T / v^T current-token projections (deferred mid-loop)."""
                kp = spsum.tile([128, STRIPE], F32, tag="sc")
                for k in range(8):
                    nc.tensor.matmul(
                        kp[:, 0:TOK], wk_sb[:, k, 128 * m : 128 * m + 128],
                        xT_bf[:, k, :], start=(k == 0), stop=(k == 7),
                    )
                nc.scalar.copy(out=kTc[:, m, :], in_=kp[:, 0:TOK])
                vp = spsum.tile([128, STRIPE], F32, tag="sc")
                for k in range(8):
                    nc.tensor.matmul(
                        vp[:, 0:TOK], wv_sb[:, k, 128 * m : 128 * m + 128],
                        xT_bf[:, k, :], start=(k == 0), stop=False,
                    )
                nc.tensor.matmul(
                    vp[:, 0:TOK], bv_sb[0:1, 128 * m : 128 * m + 128], ones_row,
                    start=False, stop=True,
                )
                nc.scalar.copy(out=vT_sb[:, m, :], in_=vp[:, 0:TOK])

            def vcur_part(m, b0):
                for b in range(b0, b0 + 4):
                    vn_ps = trpsum.tile([128, STRIPE], F32, tag="tr")
                    _trT(nc, vn_ps[0:Q, 0:128],
                         vT_sb[:, m, Q * b : Q * b + Q], ident_r)
                    nc.vector.tensor_copy(
                        v_cur[:, b, 128 * m : 128 * m + 128], vn_ps[0:Q, 0:128]
                    )

            def fin_a(fb, fo_sb):
                """wv^T extraction for batch fb (deferred to S==0 of fb+1)."""
                t_ps = ftrp.tile([128, STRIPE], F32, tag="ftr")
                for u in range(2):
                    _trF(nc, t_ps[:, 128 * u : 128 * u + 128],
                         fo_sb[:, 128 * u : 128 * u + 128], identity)
                for u in range(2):
                    nc.scalar.copy(
                        out=wvT[0:64, u, Q * fb : Q * fb + Q],
                        in_=t_ps[0:64, 128 * u + 64 * u : 128 * u + 64 * u + Q],
                    )
                    nc.vector.tensor_copy(
                        wvT[64:128, u, Q * fb : Q * fb + Q],
                        t_ps[64:128, 128 * u + 64 * u + Q :
                             128 * u + 64 * u + 2 * Q],
                    )

            def fin_b(fb):
                """y projection + transposed store (deferred to S==1). The
                last batch streams out in halves on the idle SP queue to
                shorten the post-stream tail."""
                last = fb == NB - 1
                ytp = ypsum.tile([128, 256], F32, tag="yt")
                yT_b = work.tile([128, 256], BF16, tag="yT_b")
                for hf in range(2):
                    for mo in range(4 * hf, 4 * hf + 4):
                        for k in range(2):
                            nc.tensor.matmul(
                                ytp[:, 32 * mo : 32 * mo + 32],
                                wo_sb[:, k, 128 * mo : 128 * mo + 128],
                                wvT[:, k, Q * fb : Q * fb + Q],
                                start=(k == 0), stop=(k == 1),
                            )
                    if last:
                        nc.scalar.copy(
                            out=yT_b[:, 128 * hf : 128 * hf + 128],
                            in_=ytp[:, 128 * hf : 128 * hf + 128],
                        )
                        nc.sync.dma_start(
                            out=y_d[fb, :, 128 * hf : 128 * hf + 128],
                            in_=yT_b[:, 128 * hf : 128 * hf + 128],
                        )
                if not last:
                    nc.scalar.copy(out=yT_b, in_=ytp)
                    nc.scalar.dma_start(out=y_d[fb], in_=yT_b)

            o_ps = None
            pend_a = None
            pend_b = None
            for g in range(NG):
                b, S = divmod(g, NSTRIPE)
                if g + 2 < NG:
                    kpend[g + 2] = dma_k(g + 2)
                if g + 1 < NG:
                    nv16 = dma_v(g + 1)

                if S == 0:
                    o_ps = opsum.tile([128, NWV], F32, tag="o_ps")
                    if b == NB - 1:
                        # last batch: current-token opens the group so the
                        # post-stream tail is only the final stripe's W@V
                        cur_attn(b, o_ps, True)

                # scores^T for stripe g via kt prepared last iteration
                scT = spsum.tile([128, STRIPE], F32, tag="sc")
                for jj in range(4):
                    for m in range(2):
                        nc.tensor.matmul(
                            scT[:, 128 * jj + 64 * m : 128 * jj + 64 * m + 64],
                            kt[:, m, 128 * jj : 128 * jj + 128],
                            qbd[:, 2 * b + m, :],
                            start=True, stop=True,
                        )
                wT = work.tile([128, STRIPE], F32R, tag="wT")
                if g == NG - 1:
                    nc.scalar.activation(wT[:, 0:384], scT[:, 0:384],
                                         Exp, scale=SCALE)
                    nc.scalar.activation(wT[:, 384:512], scT[:, 384:512],
                                         Exp, scale=SCALE)
                else:
                    nc.scalar.activation(wT, scT, Exp, scale=SCALE)

                # prepare next stripe's kt while exp runs
                if g + 1 < NG:
                    kt = prep_kt(kpend.pop(g + 1))

                if S == 0 and pend_a is not None:
                    fin_a(*pend_a)
                    pend_b, pend_a = pend_a[0], None
                if S == 1 and pend_b is not None:
                    fin_b(pend_b)
                    pend_b = None

                # stage-A leftovers, interleaved once their weights land
                if g == 1:
                    nc.vector.memset(v_cur[:, :, 256:258].bitcast(F32), 1.0)
                    nc.scalar.copy(out=xT_bf, in_=xT)
                if g == 2:
                    proj_kv(0)
                if g == 3:
                    proj_kv(1)
                if g == 4:
                    vcur_part(0, 0)
                    vcur_part(0, 4)
                if g == 5:
                    vcur_part(1, 0)
                    vcur_part(1, 4)

                last_b = b == NB - 1
                for jj in range(4):
                    nc.tensor.matmul(
                        o_ps,
                        wT[:, 128 * jj : 128 * jj + 128],
                        v16[:, jj, 0:NWV],
                        start=(S == 0 and jj == 0 and not last_b),
                        stop=(last_b and S == NSTRIPE - 1 and jj == 3),
                        skip_group_check=True,
                    )
                if g + 1 < NG:
                    v16 = nv16

                if S != NSTRIPE - 1:
                    continue

                # current-token attention closes the accumulation group: off
                # the stripe cadence, so batch boundaries stay smooth
                if b != NB - 1:
                    cur_attn(b, o_ps, False)

                # normalize; the rest of the finalize is deferred into the
                # next batch's first two stripe iterations so it never
                # blocks stripe work on the in-order PE queue.
                recip = work.tile([128, 1], F32, tag="recip")
                nc.vector.reciprocal(recip, o_ps[:, 256:257])
                o_sb = work.tile([128, 256], F32, tag="o_sb")
                nc.vector.tensor_scalar_mul(o_sb, o_ps[:, 0:256], recip)
                pend_a = (b, o_sb)

            fin_a(*pend_a)
            fin_b(pend_a[0])


_NC_CACHE = None


def _get_nc():
    global _NC_CACHE
    if _NC_CACHE is None:
        _NC_CACHE = _build_kernel()
    return _NC_CACHE


def kernel(**inputs):
    x = np.asarray(inputs["x"], dtype=np.float32)
    ck = np.asarray(inputs["cache_k"], dtype=np.float32)
    cv = np.asarray(inputs["cache_v"], dtype=np.float32)
    Wq = np.asarray(inputs["Wq"], dtype=np.float32)
    Wk = np.asarray(inputs["Wk"], dtype=np.float32)
    Wv = np.asarray(inputs["Wv"], dtype=np.float32)
    Wo = np.asarray(inputs["Wo"], dtype=np.float32)
    bq = np.asarray(inputs["bq"], dtype=np.float32)
    bv = np.asarray(inputs["bv"], dtype=np.float32)
    bo = np.asarray(inputs["bo"], dtype=np.float32)

    nc = _get_nc()
    in_maps = []
    for c in range(NCORES):
        dp, tp = divmod(c, NTP)
        bs = slice(NB * dp, NB * dp + NB)
        ds = slice(DSL * tp, DSL * tp + DSL)
        in_maps.append({
            "x": np.ascontiguousarray(x[bs].reshape(TOK, D)),
            "cache_k": np.ascontiguousarray(ck[bs, :, ds]),
            "cache_v": np.ascontiguousarray(cv[bs, :, ds]),
            "Wq": np.ascontiguousarray(Wq[:, ds]),
            "Wk": np.ascontiguousarray(Wk[:, ds]).astype(ml_dtypes.bfloat16),
            "Wv": np.ascontiguousarray(Wv[:, ds]).astype(ml_dtypes.bfloat16),
            "Wo": np.ascontiguousarray(Wo[ds, :]).astype(ml_dtypes.bfloat16),
            "bq": np.ascontiguousarray(bq[ds]),
            "bv": np.ascontiguousarray(bv[ds]),
        })

    res = run_bass_kernel_spmd(nc, in_maps, core_ids=list(range(NCORES)))
    global _LAST_RESULT
    _LAST_RESULT = res
    y = np.zeros((B, Q, D), dtype=np.float32)
    for c in range(NCORES):
        dp = c // NTP
        # stored transposed: [b, dim%128, mo, q] -> [b, q, mo*128 + dim%128]
        yt = np.asarray(res.results[c]["y"], dtype=np.float32).reshape(NB, 128, 8, Q)
        y[NB * dp : NB * dp + NB] += np.transpose(yt, (0, 3, 2, 1)).reshape(NB, Q, D)
    y += bo
    return y


_LAST_RESULT = None
